# revision 14
# baseline (speedup 1.0000x reference)
"""Trainium2 Bass kernel for the 2-layer decoder LSTM with attention.

Strategy (8 NeuronCores, SPMD, rank-agnostic program):
  - Tensor-parallel over feature dims for the dense matmuls: each core holds a
    1/8 shard of the big weight matrices resident in SBUF (fp32), processes
    the FULL batch (256) through its shard, and exchanges activations with
    AllGather / ReduceScatter collectives (5 per step).
  - Attention is batch-local: core r owns batch rows [32r, 32r+32). Its
    encoder block (fp32, pre-transposed to [e, s]) streams from HBM each
    step; scores run on the PE (col-packed M=1 matmuls), softmax on DVE/ACT,
    and the context reduction runs as DVE products + ACT accumulate-reduces.
  - Everything is fp32: the argmax `tokens` output flips under any reduced
    precision (measured: fp32r -> 17 flips, fp16 -> 16, bf16 -> 118; fp32 -> 0).

Activations live feature-major [feature-partition, batch-free] so LSTM gate
activations use full 128-lane ops and gate biases are per-partition ACT bias.
"""

import os
import numpy as np

import concourse.bass as bass
import concourse.bacc as bacc
import concourse.mybir as mybir
from concourse.tile import TileContext
from concourse.bass_utils import run_bass_kernel_spmd

F32 = mybir.dt.float32
I32 = mybir.dt.int32
U32 = mybir.dt.uint32
AF = mybir.ActivationFunctionType
OP = mybir.AluOpType

B, T, S, V, E, H, ENC, SUM = 256, 64, 256, 512, 512, 1024, 1024, 2048
NC = 8              # cores
BL = B // NC        # local batch rows owned for attention/outputs (32)
HS = H // NC        # h-slice per core (128)
KT = int(os.environ.get("KT", str(T)))   # unrolled steps (dev knob)
RG = [list(range(NC))]

_BUILD_CACHE = {}


def _pack_lhsT(Wt):
    """Pack W.T ([fin, fout]) into SBUF lhsT-block layout [128, nk*nm*128].

    Block (kt, mt) = Wt[kt*128:(kt+1)*128, mt*128:(mt+1)*128], stored at
    free-offset (kt*nm + mt)*128.
    """
    fin, fout = Wt.shape
    nk, nm = fin // 128, fout // 128
    return (
        Wt.reshape(nk, 128, nm, 128)
        .transpose(1, 0, 2, 3)
        .reshape(128, nk * nm * 128)
        .astype(np.float32)
        .copy()
    )


def _gate_rows(r):
    """Rows of a [4H, *] gate matrix owned by core r: 4 gates x h-slice."""
    return np.concatenate(
        [np.arange(g * H + r * HS, g * H + (r + 1) * HS) for g in range(4)]
    )


def _build(kt):
    if kt in _BUILD_CACHE:
        return _BUILD_CACHE[kt]
    nc = bacc.Bacc("TRN2", target_bir_lowering=False, debug=False)
    NTOK = B * kt          # embeddings for the FULL batch on every core
    NTT = NTOK // 128      # token tiles (B*kt multiple of 128)

    d = {}
    def din(name, shape, dtype=F32):
        d[name] = nc.dram_tensor(name, list(shape), dtype, kind="ExternalInput")
        return d[name]

    din("enc_tT", [BL, ENC, S])                 # [b, e, s] fp32 (streamed)
    din("ytb", [NTOK], I32)                     # tokens, (t, global-b) order
    din("emb_t", [V, E])
    din("summary_t", [128, (SUM // 128) * B])   # packed rhs tiles [128, 16*256]
    din("w_init", [128, (SUM // 128) * 4 * 128])  # lhsT blocks (mt: h0,h1,c0,c1)
    din("wci_e", [128, 4 * 4 * 128])
    din("wci_x", [128, 8 * 4 * 128])
    din("wih0", [128, 4 * 4 * 128])
    din("whh0", [128, 8 * 4 * 128])
    din("wih1", [128, 8 * 4 * 128])
    din("whh1", [128, 8 * 4 * 128])
    din("wq_t", [128, 2 * H])
    din("wco", [128, 16 * 1 * 128])
    din("wout_t", [128, V])
    din("bci", [128, 4])
    din("bg0", [128, 4])
    din("bg1", [128, 4])
    din("binit", [128, 4])
    din("bco", [128, 1])
    din("bq8", [128, H])
    din("bout8", [128, V])
    din("ident", [128, 128])
    din("sel", [128, 4 * 128])

    toks_o = nc.dram_tensor("toks_o", [BL, kt], U32, kind="ExternalOutput")
    lins_o = nc.dram_tensor("lins_o", [BL, kt, V], F32, kind="ExternalOutput")

    dbg = os.environ.get("KDBG", "0") == "1"
    dbg_o = {}
    if dbg:
        for nm, shape in [
            ("xT0", [128, 4 * B]), ("h0s0", [128, B]), ("c0s0", [128, B]),
            ("h1s0", [128, B]), ("q0", [BL, H]), ("sc0", [BL, S]),
            ("at0", [BL, S]), ("crT0", [128, 8 * BL]), ("lin0", [128, B]),
        ]:
            dbg_o[nm] = nc.dram_tensor(nm, shape, F32, kind="ExternalOutput")

    with TileContext(nc) as tc:
        with (
            tc.tile_pool(name="wp", bufs=1) as wp,
            tc.tile_pool(name="st", bufs=1) as st,
            tc.tile_pool(name="wk", bufs=2) as wk,
            tc.tile_pool(name="ck", bufs=2) as ck,
            tc.tile_pool(name="psG", bufs=2, space="PSUM") as psG,
            tc.tile_pool(name="psS", bufs=2, space="PSUM") as psS,
            tc.tile_pool(name="dcl", bufs=2, space="DRAM") as dcl,
            tc.tile_pool(name="dst", bufs=1, space="DRAM") as dst,
        ):
            # ---------------- resident weights ----------------
            w = {}
            for name in ["wci_e", "wci_x", "wih0", "whh0", "wih1", "whh1",
                         "wq_t", "wco", "wout_t", "bci", "bg0", "bg1",
                         "binit", "bco", "bq8", "bout8", "ident", "sel"]:
                sh = list(d[name].shape)
                t_ = wp.tile(sh, F32, tag=name)
                nc.sync.dma_start(out=t_[:], in_=d[name][:])
                w[name] = t_

            def blk(wt, kt_, mt_, nm_):
                return wt[:, (kt_ * nm_ + mt_) * 128:(kt_ * nm_ + mt_ + 1) * 128]

            # ---------------- persistent state ----------------
            h0s = st.tile([128, B], F32, tag="h0s")
            c0s = st.tile([128, B], F32, tag="c0s")
            h1s = st.tile([128, B], F32, tag="h1s")
            c1s = st.tile([128, B], F32, tag="c1s")
            h0f = st.tile([128, 8 * B], F32, tag="h0f")   # full h0 [h-feat, b]
            h1f = st.tile([128, 8 * B], F32, tag="h1f")
            crf = st.tile([128, 8 * B], F32, tag="crf")   # full cross
            xT = st.tile([128, 4 * B], F32, tag="xT")
            qT = st.tile([128, 8 * BL], F32, tag="qT")    # q feature-major
            crl = st.tile([128, 8 * BL], F32, tag="crl")  # local crossT
            toks = st.tile([BL, kt], U32, tag="toks")
            ytb_sb = st.tile([128, NTT], I32, tag="ytb")
            nc.sync.dma_start(
                out=ytb_sb[:], in_=d["ytb"][:].rearrange("(k p) -> p k", p=128))

            # ---------------- embedding gather -> embT in DRAM ----------------
            embT_d = dst.tile([128, 4, NTOK], F32, tag="embT")
            for k in range(NTT):
                eg = wk.tile([128, E], F32, tag="embG", bufs=1)
                nc.gpsimd.indirect_dma_start(
                    out=eg[:], out_offset=None, in_=d["emb_t"][:],
                    in_offset=bass.IndirectOffsetOnAxis(ap=ytb_sb[:, k:k + 1], axis=0),
                )
                for et in range(4):
                    pt = psS.tile([128, 128], F32, tag="ab")
                    nc.tensor.matmul(pt[:], lhsT=eg[:, et * 128:(et + 1) * 128],
                                     rhs=w["ident"][:], start=True, stop=True)
                    es = wk.tile([128, 128], F32, tag="embTs", bufs=1)
                    nc.vector.tensor_copy(es[:], pt[:])
                    nc.sync.dma_start(
                        out=embT_d[:, et, k * 128:(k + 1) * 128], in_=es[:])

            # ---------------- init state ----------------
            init_dst = [h0s, h1s, c0s, c1s]
            for mt in range(4):
                ps = psG.tile([128, B], F32, tag="g")
                for kk in range(16):
                    wi = wk.tile([128, 128], F32, tag="wi", bufs=2)
                    nc.sync.dma_start(
                        out=wi[:],
                        in_=d["w_init"][:, (kk * 4 + mt) * 128:
                                        (kk * 4 + mt + 1) * 128])
                    sm = ck.tile([128, B], F32, tag="sm", bufs=2)
                    nc.sync.dma_start(
                        out=sm[:], in_=d["summary_t"][:, kk * B:(kk + 1) * B])
                    nc.tensor.matmul(
                        ps[:], lhsT=wi[:], rhs=sm[:],
                        start=(kk == 0), stop=(kk == 15))
                nc.scalar.activation(init_dst[mt][:], ps[:], AF.Tanh,
                                     bias=w["binit"][:, mt:mt + 1])
            nc.vector.memset(crf[:], 0.0)

            # ---------------- helpers ----------------
            def allgather_slice(src_tile, dstf, tag):
                ain = dcl.tile([128, B], F32, tag=tag + "i")
                aout = dcl.tile([NC, 128, B], F32, tag=tag + "o",
                                addr_space="Shared")
                nc.sync.dma_start(out=ain[:], in_=src_tile[:])
                nc.gpsimd.collective_compute(
                    "AllGather", OP.bypass, ins=[ain[:]], outs=[aout[:]],
                    replica_groups=RG)
                for kk in range(NC):
                    nc.sync.dma_start(out=dstf[:, kk * B:(kk + 1) * B],
                                      in_=aout[kk])

            GATE_F = [AF.Sigmoid, AF.Sigmoid, AF.Tanh, AF.Sigmoid]

            def cell_combine(acts, cs, hs_):
                si, sf, tg, so = acts
                t1 = wk.tile([128, B], F32, tag="t1", bufs=1)
                nc.vector.tensor_tensor(out=t1[:], in0=sf[:], in1=cs[:], op=OP.mult)
                t2 = wk.tile([128, B], F32, tag="t2", bufs=1)
                nc.vector.tensor_tensor(out=t2[:], in0=si[:], in1=tg[:], op=OP.mult)
                nc.vector.tensor_tensor(out=cs[:], in0=t1[:], in1=t2[:], op=OP.add)
                tc_ = wk.tile([128, B], F32, tag="tc", bufs=1)
                nc.scalar.activation(tc_[:], cs[:], AF.Tanh)
                nc.vector.tensor_tensor(out=hs_[:], in0=so[:], in1=tc_[:], op=OP.mult)

            # initial AGs so step 0 sees full h0/h1
            allgather_slice(h0s, h0f, "agh0")
            allgather_slice(h1s, h1f, "agh1")

            # ---------------- time loop ----------------
            for t in range(kt):
                # --- x_t = tanh(W_ci @ [emb_t; cross_{t-1}]) ---
                ebt = wk.tile([128, 4, B], F32, tag="ebt", bufs=1)
                nc.sync.dma_start(out=ebt[:], in_=embT_d[:, :, t * B:(t + 1) * B])
                for mt in range(4):
                    ps = psG.tile([128, B], F32, tag="g")
                    for kk in range(4):
                        nc.tensor.matmul(
                            ps[:], lhsT=blk(w["wci_e"], kk, mt, 4),
                            rhs=ebt[:, kk, :], start=(kk == 0), stop=False)
                    for kk in range(8):
                        nc.tensor.matmul(
                            ps[:], lhsT=blk(w["wci_x"], kk, mt, 4),
                            rhs=crf[:, kk * B:(kk + 1) * B],
                            start=False, stop=(kk == 7))
                    nc.scalar.activation(xT[:, mt * B:(mt + 1) * B], ps[:],
                                         AF.Tanh, bias=w["bci"][:, mt:mt + 1])

                # --- layer-0 gates ---
                acts0 = []
                for mt in range(4):
                    ps = psG.tile([128, B], F32, tag="g")
                    for kk in range(4):
                        nc.tensor.matmul(
                            ps[:], lhsT=blk(w["wih0"], kk, mt, 4),
                            rhs=xT[:, kk * B:(kk + 1) * B],
                            start=(kk == 0), stop=False)
                    for kk in range(8):
                        nc.tensor.matmul(
                            ps[:], lhsT=blk(w["whh0"], kk, mt, 4),
                            rhs=h0f[:, kk * B:(kk + 1) * B],
                            start=False, stop=(kk == 7))
                    a = wk.tile([128, B], F32, tag=f"a{mt}", bufs=1)
                    nc.scalar.activation(a[:], ps[:], GATE_F[mt],
                                         bias=w["bg0"][:, mt:mt + 1])
                    acts0.append(a)
                cell_combine(acts0, c0s, h0s)

                # --- layer-1 gates: Whh1 (old h1f) for 2 gates overlaps AG(h0)
                g1ps = {}
                for mt in range(2):
                    ps = psG.tile([128, B], F32, tag="g1", bufs=2)
                    g1ps[mt] = ps
                    for kk in range(8):
                        nc.tensor.matmul(
                            ps[:], lhsT=blk(w["whh1"], kk, mt, 4),
                            rhs=h1f[:, kk * B:(kk + 1) * B],
                            start=(kk == 0), stop=False)
                allgather_slice(h0s, h0f, "agh0")
                acts1 = []
                for mt in range(4):
                    if mt >= 2:
                        ps = psG.tile([128, B], F32, tag="g1", bufs=2)
                        g1ps[mt] = ps
                        for kk in range(8):
                            nc.tensor.matmul(
                                ps[:], lhsT=blk(w["whh1"], kk, mt, 4),
                                rhs=h1f[:, kk * B:(kk + 1) * B],
                                start=(kk == 0), stop=False)
                    for kk in range(8):
                        nc.tensor.matmul(
                            g1ps[mt][:], lhsT=blk(w["wih1"], kk, mt, 4),
                            rhs=h0f[:, kk * B:(kk + 1) * B],
                            start=False, stop=(kk == 7))
                    a = wk.tile([128, B], F32, tag=f"b{mt}", bufs=1)
                    nc.scalar.activation(a[:], g1ps[mt][:], GATE_F[mt],
                                         bias=w["bg1"][:, mt:mt + 1])
                    acts1.append(a)
                cell_combine(acts1, c1s, h1s)

                # --- q partial + ReduceScatter (batch-shard) ---
                rsq_i = dcl.tile([B, H], F32, tag="rsqi")
                for bt in range(2):
                    for nf in range(2):
                        ps = psG.tile([128, 512], F32, tag="g")
                        for kk, lhs in ((0, h1s), (1, c1s)):
                            nc.tensor.matmul(
                                ps[:], lhsT=lhs[:, bt * 128:(bt + 1) * 128],
                                rhs=w["wq_t"][:, kk * H + nf * 512:
                                              kk * H + (nf + 1) * 512],
                                start=(kk == 0), stop=(kk == 1))
                        qa = wk.tile([128, 512], F32, tag="qa")
                        nc.vector.tensor_tensor(
                            out=qa[:], in0=ps[:],
                            in1=w["bq8"][:, nf * 512:(nf + 1) * 512], op=OP.add)
                        nc.sync.dma_start(
                            out=rsq_i[bt * 128:(bt + 1) * 128,
                                      nf * 512:(nf + 1) * 512], in_=qa[:])
                rsq_o = dcl.tile([BL, H], F32, tag="rsqo")
                nc.gpsimd.collective_compute(
                    "ReduceScatter", OP.add, ins=[rsq_i[:]], outs=[rsq_o[:]],
                    replica_groups=RG)
                allgather_slice(h1s, h1f, "agh1")

                qsb = wk.tile([BL, H], F32, tag="qsb", bufs=1)
                nc.sync.dma_start(out=qsb[:], in_=rsq_o[:])
                qth = qsb
                nc.scalar.activation(qth[:], qsb[:], AF.Tanh)

                for kk in range(8):
                    pt = psS.tile([128, 128], F32, tag="ab")
                    nc.tensor.matmul(
                        pt[:, 0:BL], lhsT=qth[:, kk * 128:(kk + 1) * 128],
                        rhs=w["ident"][0:BL, 0:BL], start=True, stop=True)
                    nc.vector.tensor_copy(qT[:, kk * BL:(kk + 1) * BL],
                                          pt[:, 0:BL])

                # --- attention (batch-local, streamed enc) ---
                # groups of 2 b's; enc chunks split in e-halves [128, 4, S]
                for gg in range(BL // 2):
                    sps = psS.tile([128, S], F32, tag="sc")
                    chs = {}
                    for j in range(2):
                        b = gg * 2 + j
                        for hf in range(2):
                            chv = ck.tile([128, 4, S], F32, tag="ch", bufs=5)
                            chs[(j, hf)] = chv
                            nc.sync.dma_start(
                                out=chv[:],
                                in_=d["enc_tT"][b, hf * 512:(hf + 1) * 512]
                                .rearrange("(et p) s -> p et s", p=128))
                            for k4 in range(4):
                                kk = hf * 4 + k4
                                nc.tensor.matmul(
                                    sps[32 * j:32 * j + 1, :],
                                    lhsT=qT[:, kk * BL + b:kk * BL + b + 1],
                                    rhs=chv[:, k4, :],
                                    start=(kk == 0), stop=(kk == 7),
                                    tile_position=(0, 32 * j))
                    # packed softmax on [128, S] (rows 0/32 valid)
                    mx = wk.tile([128, 1], F32, tag="mx")
                    nc.vector.tensor_reduce(out=mx[:], in_=sps[:],
                                            axis=mybir.AxisListType.X, op=OP.max)
                    nmx = wk.tile([128, 1], F32, tag="nmx")
                    nc.scalar.mul(nmx[:], mx[:], -1.0)
                    ex = wk.tile([128, S], F32, tag="ex", bufs=1)
                    zs = wk.tile([128, 1], F32, tag="zs")
                    nc.scalar.activation(ex[:], sps[:], AF.Exp,
                                         bias=nmx[:, 0:1], accum_out=zs[:])
                    rz = wk.tile([128, 1], F32, tag="rz")
                    nc.vector.reciprocal(rz[:], zs[:])
                    att = wk.tile([128, S], F32, tag="att")
                    nc.vector.tensor_scalar_mul(att[:], ex[:], rz[:, 0:1])
                    for j in range(2):
                        b = gg * 2 + j
                        pb = psS.tile([128, S], F32, tag="ab")
                        nc.tensor.matmul(
                            pb[:], lhsT=w["sel"][:, j * 128:(j + 1) * 128],
                            rhs=att[:], start=True, stop=True)
                        abc = wk.tile([128, S], F32, tag="abc")
                        nc.vector.tensor_copy(abc[:], pb[:])
                        dmy = wk.tile([128, 1], F32, tag="dmy")
                        for kk in range(8):
                            prod = wk.tile([128, S], F32, tag="prod", bufs=4)
                            nc.vector.tensor_tensor(
                                out=prod[:], in0=chs[(j, kk // 4)][:, kk % 4, :],
                                in1=abc[:], op=OP.mult)
                            nc.scalar.activation(
                                dmy.broadcast_to([128, S]), prod[:], AF.Copy,
                                accum_out=crl[:, kk * BL + b:kk * BL + b + 1])
                    if dbg and t == 0 and gg == 0:
                        nc.sync.dma_start(
                            out=dbg_o["sc0"][0:2, :],
                            in_=sps[:].rearrange("(j q) s -> j q s", q=32)[0:2, 0, :])
                        nc.sync.dma_start(
                            out=dbg_o["at0"][0:2, :],
                            in_=att[:].rearrange("(j q) s -> j q s", q=32)[0:2, 0, :])

                # --- AllGather cross ---
                agc_i = dcl.tile([ENC, BL], F32, tag="agci")
                nc.sync.dma_start(
                    out=agc_i[:].rearrange("(et p) b -> p et b", p=128),
                    in_=crl[:].rearrange("p (et b) -> p et b", et=8))
                agc_o = dcl.tile([NC, ENC, BL], F32, tag="agco",
                                 addr_space="Shared")
                nc.gpsimd.collective_compute(
                    "AllGather", OP.bypass, ins=[agc_i[:]], outs=[agc_o[:]],
                    replica_groups=RG)
                for et in range(8):
                    nc.sync.dma_start(
                        out=crf[:, et * B:(et + 1) * B].rearrange(
                            "p (r b) -> p r b", r=NC),
                        in_=agc_o[:, et * 128:(et + 1) * 128, :].rearrange(
                            "r p b -> p r b"))

                # --- lin = tanh(Wco @ [h1f; crf]), logits, RS, argmax ---
                lps = psG.tile([128, B], F32, tag="g")
                for kk in range(16):
                    rhs = (h1f[:, kk * B:(kk + 1) * B] if kk < 8
                           else crf[:, (kk - 8) * B:(kk - 7) * B])
                    nc.tensor.matmul(lps[:], lhsT=blk(w["wco"], kk, 0, 1),
                                     rhs=rhs, start=(kk == 0), stop=(kk == 15))
                lin = wk.tile([128, B], F32, tag="lin", bufs=1)
                nc.scalar.activation(lin[:], lps[:], AF.Tanh,
                                     bias=w["bco"][:, 0:1])
                rsl_i = dcl.tile([B, V], F32, tag="rsli")
                for bt in range(2):
                    ps = psG.tile([128, V], F32, tag="g")
                    nc.tensor.matmul(ps[:], lhsT=lin[:, bt * 128:(bt + 1) * 128],
                                     rhs=w["wout_t"][:], start=True, stop=True)
                    la = wk.tile([128, V], F32, tag="la", bufs=1)
                    nc.vector.tensor_tensor(out=la[:], in0=ps[:],
                                            in1=w["bout8"][:], op=OP.add)
                    nc.sync.dma_start(out=rsl_i[bt * 128:(bt + 1) * 128, :],
                                      in_=la[:])
                rsl_o = dcl.tile([BL, V], F32, tag="rslo")
                nc.gpsimd.collective_compute(
                    "ReduceScatter", OP.add, ins=[rsl_i[:]], outs=[rsl_o[:]],
                    replica_groups=RG)
                lsb = wk.tile([BL, V], F32, tag="lsb")
                nc.sync.dma_start(out=lsb[:], in_=rsl_o[:])
                nc.sync.dma_start(out=lins_o[:, t, :], in_=lsb[:])
                mx8 = wk.tile([BL, 8], F32, tag="mx8")
                nc.vector.max(mx8[:], lsb[:])
                ix8 = wk.tile([BL, 8], U32, tag="ix8")
                nc.vector.max_index(ix8[:], mx8[:], lsb[:])
                nc.vector.tensor_copy(toks[:, t:t + 1], ix8[:, 0:1])

                if dbg and t == 0:
                    nc.sync.dma_start(out=dbg_o["xT0"][:], in_=xT[:])
                    nc.sync.dma_start(out=dbg_o["h0s0"][:], in_=h0s[:])
                    nc.sync.dma_start(out=dbg_o["c0s0"][:], in_=c0s[:])
                    nc.sync.dma_start(out=dbg_o["h1s0"][:], in_=h1s[:])
                    nc.sync.dma_start(out=dbg_o["q0"][:], in_=qth[:])
                    nc.sync.dma_start(out=dbg_o["crT0"][:], in_=crl[:])
                    nc.sync.dma_start(out=dbg_o["lin0"][:], in_=lin[:])

            nc.sync.dma_start(out=toks_o[:], in_=toks[:])

    nc.compile()
    _BUILD_CACHE[kt] = nc
    return nc


def _prep_core(inputs, r, kt):
    """Host-side slicing/packing of full inputs for core r."""
    f32 = np.float32
    g = lambda n: np.asarray(inputs[n], dtype=f32)
    y = np.asarray(inputs["y"])
    enc = g("encoder_embed")
    summ = g("encoder_summary")
    myb = slice(r * BL, (r + 1) * BL)
    gr = _gate_rows(r)

    m = {}
    m["enc_tT"] = np.ascontiguousarray(enc[myb].transpose(0, 2, 1))
    m["ytb"] = y[:, :kt].T.reshape(-1).astype(np.int32)   # (t, global-b)
    m["emb_t"] = g("emb_table")
    m["summary_t"] = (
        summ.T.reshape(SUM // 128, 128, B).transpose(1, 0, 2).reshape(128, -1)
        .astype(f32).copy())

    W_init = g("W_init")   # [4H, SUM]; row blocks: h0, h1, c0, c1
    rows = np.concatenate([
        np.arange(0 * H + r * HS, 0 * H + (r + 1) * HS),
        np.arange(1 * H + r * HS, 1 * H + (r + 1) * HS),
        np.arange(2 * H + r * HS, 2 * H + (r + 1) * HS),
        np.arange(3 * H + r * HS, 3 * H + (r + 1) * HS),
    ])
    m["w_init"] = _pack_lhsT(np.ascontiguousarray(W_init[rows].T))
    m["binit"] = g("b_init")[rows].reshape(4, 128).T.copy()

    W_ci = g("W_ci")      # [E, E+ENC], full output on every core
    m["wci_e"] = _pack_lhsT(np.ascontiguousarray(W_ci[:, :E].T))
    m["wci_x"] = _pack_lhsT(np.ascontiguousarray(W_ci[:, E:].T))
    m["bci"] = g("b_ci").reshape(4, 128).T.copy()

    m["wih0"] = _pack_lhsT(np.ascontiguousarray(g("Wih0")[gr].T))
    m["whh0"] = _pack_lhsT(np.ascontiguousarray(g("Whh0")[gr].T))
    m["wih1"] = _pack_lhsT(np.ascontiguousarray(g("Wih1")[gr].T))
    m["whh1"] = _pack_lhsT(np.ascontiguousarray(g("Whh1")[gr].T))
    m["bg0"] = (g("bih0") + g("bhh0"))[gr].reshape(4, 128).T.copy()
    m["bg1"] = (g("bih1") + g("bhh1"))[gr].reshape(4, 128).T.copy()

    # q contraction slice = my local h1 rows ++ my local c1 rows
    Wqt = g("Wq").T       # [2H, ENC-out] rows: h1 feats 0..1023, c1 feats ..
    wqt = np.concatenate([
        Wqt[r * HS:(r + 1) * HS, :],           # my h1 slice rows
        Wqt[H + r * HS:H + (r + 1) * HS, :],   # my c1 slice rows
    ])                                          # [256, 1024]
    m["wq_t"] = wqt.reshape(2, 128, H).transpose(1, 0, 2).reshape(128, -1).copy()
    m["bq8"] = np.tile(g("bq")[None, :] / NC, (128, 1)).astype(f32)

    Wco = g("Wco")        # [H, H+ENC]; cat is [h1, cross]
    m["wco"] = _pack_lhsT(np.ascontiguousarray(Wco[r * HS:(r + 1) * HS, :].T))
    m["bco"] = g("bco")[r * HS:(r + 1) * HS].reshape(128, 1).copy()

    m["wout_t"] = np.ascontiguousarray(g("Wout").T[r * HS:(r + 1) * HS, :])
    m["bout8"] = np.tile(g("bout")[None, :] / NC, (128, 1)).astype(f32)

    m["ident"] = np.eye(128, dtype=f32)
    sel = np.zeros((128, 4 * 128), f32)
    for j in range(4):
        sel[32 * j, j * 128:(j + 1) * 128] = 1.0
    m["sel"] = sel
    return m


def kernel(**inputs):
    kt = KT
    nc = _build(kt)
    in_maps = [_prep_core(inputs, r, kt) for r in range(NC)]
    trace = os.environ.get("KTRACE", "0") == "1"
    res = run_bass_kernel_spmd(nc, in_maps, core_ids=list(range(NC)),
                               trace=trace)
    toks = np.concatenate([res.results[r]["toks_o"] for r in range(NC)], axis=0)
    lins = np.concatenate([res.results[r]["lins_o"] for r in range(NC)], axis=0)
    kernel._last = res
    return toks.astype(np.int32), lins
